# revision 65
# baseline (speedup 1.0000x reference)
"""Multi-head attention (B=1, S=2048, H=1024, NH=16) on 8 trn2 NeuronCores.

Sharding: head-parallel. Core c owns heads {2c, 2c+1} (= 128 of the 1024
hidden dims). Each core computes its Q/K/V projection slices, the full
attention for its 2 heads, and a full-width partial of the output
projection (contraction over its 128 context dims). Host sums the 8
partials and adds the (host-folded) biases.

Schedule notes (from TimelineSim iteration; ~134us modeled vs 157us v1):
  - Real TRN2 GpSimd cannot access PSUM, and nearly all elementwise work
    here reads PSUM, so DVE carries the mask multiplies (2x [128,1024]
    TensorTensor per (h,j), ~2.4us/iter = the cadence) and Act carries
    exp plus most PSUM->SBUF evictions in its gaps. GpSimd only memsets.
  - Writes from different engines into one tile serialize (tile-level
    WAW), so every multi-writer tile is split: qT/kT per 512-panel,
    vaug per key-chunk, sm/et single-writer.
  - Q/K biases fold into the eviction (Act Identity+bias / DVE
    tensor_scalar_add); no bias matmuls.
  - K projection in 4 token-panels: p0 prefetched inside the Q-proj DMA
    gaps (PSUM misc bank spans both phases), p1-3 interleaved at j=3,7,11.
  - PE pstate warmup: junk matmuls on a memset tile keep the dispatch
    stream dense through the DMA-gated projection so the model's p-state
    stays at 2.4GHz.
  - DMAs are emitted in deadline order (mask in j-pair chunks, V in
    quarter-chunks) because the 16.3MB input stream (~47us) paces h0.
  - h0's transposes are deferred into the h1 loop; h1's transposes and
    the output projection run after the attention pools close, in
    dedicated PSUM pools (tp x4 banks, y x3) so the MM->evict chain
    pipelines; output DMA per (nn, seq-half).

Precision: all matmuls bf16 with fp32 PSUM accumulation. The 0/1 mask is
stored fp8-e4m3 (exact, halves its bandwidth). Softmax runs without
max-subtraction: the exponent is (q.k/8)*M ~ N(0, 0.33^2), so exp never
overflows.
"""

import math
from collections import deque

import numpy as np
import ml_dtypes

BF16 = ml_dtypes.bfloat16
FP8 = ml_dtypes.float8_e4m3
S, H, NH, DK = 2048, 1024, 16, 64
NCORES = 8
HPC = NH // NCORES          # heads per core = 2
DPC = HPC * DK              # head dims per core = 128
KC = H // 128               # contraction chunks = 8
TP = S // 512               # 512-wide token panels = 4
JC = S // 128               # 128-wide key chunks = 16
VA = DK + 1                 # v columns + ones column = 65

_CACHE = {}


def _oslc(ic):
    """o_ps column offset for ic-th 65-wide slice: 7 slices per 512-fp32
    PSUM bank so no matmul crosses a bank boundary."""
    b, r = divmod(ic, 7)
    return b * 512 + r * VA


def _build_program():
    """Build + compile the (identical) per-core Bass program."""
    from contextlib import ExitStack

    import concourse.bacc as bacc
    import concourse.tile as tile
    from concourse import mybir
    import concourse.bass as bass_mod

    dt = mybir.dt
    AF = mybir.ActivationFunctionType
    f8 = dt.float8e4

    nc = bacc.Bacc("TRN2", target_bir_lowering=False, debug=False)

    qT_d = nc.dram_tensor("qT", [H, S], dt.bfloat16, kind="ExternalInput").ap()
    kT_d = nc.dram_tensor("kT", [H, S], dt.bfloat16, kind="ExternalInput").ap()
    vT_d = nc.dram_tensor("vT", [H, S], dt.bfloat16, kind="ExternalInput").ap()
    maskT_d = nc.dram_tensor("maskT", [S, S], f8, kind="ExternalInput").ap()
    wq_d = nc.dram_tensor("wq", [128, KC * DPC], dt.bfloat16, kind="ExternalInput").ap()
    wk_d = nc.dram_tensor("wk", [128, KC * DPC], dt.bfloat16, kind="ExternalInput").ap()
    wv_d = nc.dram_tensor("wv", [128, KC * DPC], dt.bfloat16, kind="ExternalInput").ap()
    wo_d = nc.dram_tensor("wo", [DPC, H], dt.bfloat16, kind="ExternalInput").ap()
    bq_d = nc.dram_tensor("bq", [DPC, 1], dt.float32, kind="ExternalInput").ap()
    bk_d = nc.dram_tensor("bk", [DPC, 1], dt.float32, kind="ExternalInput").ap()
    id_d = nc.dram_tensor("ident", [128, 128], dt.bfloat16, kind="ExternalInput").ap()
    yT_d = nc.dram_tensor("yT", [H, S], dt.bfloat16, kind="ExternalOutput").ap()

    with tile.TileContext(nc) as tc, ExitStack() as ctx:
        cp = ctx.enter_context(tc.tile_pool(name="const", bufs=1))
        sm_p = ctx.enter_context(tc.tile_pool(name="sm", bufs=3))
        e_p = ctx.enter_context(tc.tile_pool(name="ex", bufs=4))
        ot_p = ctx.enter_context(tc.tile_pool(name="otok", bufs=2))
        rc_p = ctx.enter_context(tc.tile_pool(name="recip", bufs=3))
        w_sb = {}
        for name, d in (("wq", wq_d), ("wk", wk_d), ("wv", wv_d)):
            w_sb[name] = cp.tile([128, KC * DPC], dt.bfloat16, tag=name, name=name)
        wo_sb = cp.tile([128, H], dt.bfloat16, tag="wo")
        bq_sb = cp.tile([DPC, 1], dt.float32, tag="bq")
        bk_sb = cp.tile([DPC, 1], dt.float32, tag="bk")
        qT_p = [cp.tile([128, 512], dt.bfloat16, tag=f"qT{p}", name=f"qT{p}")
                for p in range(TP)]
        kT_p = [cp.tile([128, 512], dt.bfloat16, tag=f"kT{p}", name=f"kT{p}")
                for p in range(TP)]
        vaug_t = [cp.tile([128, HPC * VA], dt.bfloat16, tag=f"vg{t}",
                          name=f"vg{t}") for t in range(JC)]
        ident = cp.tile([128, 128], dt.bfloat16, tag="ident")
        oT_sb = [cp.tile([128, 512], dt.bfloat16, tag=f"oTp{p}", name=f"oTp{p}")
                 for p in range(TP)]
        mask_sb = cp.tile([128, JC * S], f8, tag="mask")
        kin = [cp.tile([128, KC * 512], dt.bfloat16, tag=f"kin{p}", name=f"kin{p}")
               for p in range(TP)]
        vinq = [cp.tile([128, KC * 512], dt.bfloat16, tag=f"vin{qt}",
                        name=f"vin{qt}") for qt in range(4)]
        y_sb = [cp.tile([128, S], dt.bfloat16, tag=f"ysb{nn}", name=f"ysb{nn}")
                for nn in range(KC)]

        def dma_mask2(j2):
            nc.sync.dma_start(
                out=mask_sb[:, j2 * 2 * S : (j2 + 1) * 2 * S].rearrange(
                    "p (a i) -> p a i", a=2
                ),
                in_=maskT_d[j2 * 256 : (j2 + 1) * 256, :].rearrange(
                    "(a p) i -> p a i", p=128
                ),
            )

        def dma_kin(p):
            nc.sync.dma_start(
                out=kin[p].rearrange("p (c i) -> p c i", c=KC),
                in_=kT_d[:, p * 512 : (p + 1) * 512].rearrange(
                    "(c p) i -> p c i", p=128
                ),
            )

        def dma_vq(qt):
            nc.sync.dma_start(
                out=vinq[qt].rearrange("p (c i) -> p c i", c=KC),
                in_=vT_d[:, qt * 512 : (qt + 1) * 512].rearrange(
                    "(c p) i -> p c i", p=128
                ),
            )

        # ---- Q projection (+ all input DMAs, emitted in deadline order) ----
        pm = ctx.enter_context(tc.tile_pool(name="ps_misc", bufs=1, space="PSUM"))

        def kproj_mms(ps, p, kks):
            for kk in kks:
                nc.tensor.matmul(
                    ps,
                    lhsT=w_sb["wk"][:, kk * DPC : (kk + 1) * DPC],
                    rhs=kin[p][:, kk * 512 : (kk + 1) * 512],
                    start=(kk == 0),
                    stop=(kk == KC - 1),
                )

        def kproj(p):
            ps = pm.tile([128, 512], dt.float32, tag="misc", name=f"pk{p}")
            kproj_mms(ps, p, range(KC))
            nc.scalar.activation(kT_p[p], ps, AF.Identity, bias=bk_sb)

        with tc.tile_pool(name="ps_q", bufs=1, space="PSUM") as pq, \
             tc.tile_pool(name="xin", bufs=3) as xin_p:
            nc.sync.dma_start(out=w_sb["wq"], in_=wq_d)
            nc.sync.dma_start(out=w_sb["wk"], in_=wk_d)
            xq = []

            def dma_xq(hf):
                for cc in range(2):
                    xt = xin_p.tile([128, S], dt.bfloat16, tag="xq",
                                    name=f"xq{2 * hf + cc}")
                    nc.sync.dma_start(
                        out=xt,
                        in_=qT_d[hf * 256 + cc * 128
                                 : hf * 256 + (cc + 1) * 128, :],
                    )
                    xq.append(xt)

            dma_xq(0)
            dma_kin(0)
            dma_xq(1)
            dma_mask2(0)
            nc.sync.dma_start(out=bq_sb, in_=bq_d)
            nc.sync.dma_start(out=bk_sb, in_=bk_d)
            dma_xq(2)
            dma_xq(3)
            nc.sync.dma_start(out=w_sb["wv"], in_=wv_d)
            dma_vq(0)
            dma_mask2(1)
            dma_kin(1)
            dma_vq(1)
            dma_mask2(2)
            nc.sync.dma_start(out=ident, in_=id_d)
            dma_mask2(3)
            dma_kin(2)
            dma_mask2(4)
            dma_vq(2)
            dma_mask2(5)
            dma_kin(3)
            dma_mask2(6)
            dma_vq(3)
            dma_mask2(7)
            nc.sync.dma_start(out=wo_sb, in_=wo_d)

            psl = [pq.tile([128, 512], dt.float32, tag=f"pq{p}", name=f"pq{p}")
                   for p in range(TP)]
            warm_in = cp.tile([128, 256], dt.bfloat16, tag="warm")
            nc.vector.memset(warm_in, 0.0)
            junk = pq.tile([128, 256], dt.float32, tag="junk")

            def warm(n):
                for _ in range(n):
                    nc.tensor.matmul(
                        junk, lhsT=warm_in[:, 0:128], rhs=warm_in,
                        start=True, stop=True,
                    )

            warm(40)
            for kk in range(KC):
                for p in range(TP):
                    nc.tensor.matmul(
                        psl[p],
                        lhsT=w_sb["wq"][:, kk * DPC : (kk + 1) * DPC],
                        rhs=xq[kk][:, p * 512 : (p + 1) * 512],
                        start=(kk == 0),
                        stop=(kk == KC - 1),
                    )
                if kk % 2 == 1 and kk < 7:
                    warm(6)
                if kk == 3:
                    kproj(0)
            for p in range(TP):
                if p % 2 == 0:
                    nc.scalar.activation(
                        qT_p[p], psl[p], AF.Identity, bias=bq_sb,
                    )
                else:
                    nc.vector.tensor_scalar_add(qT_p[p], psl[p], bq_sb)

        # ---- attention (K panels + V projection interleaved) ----
        # PSUM: misc (kproj/vproj/tp) 1 bank + s 2x2 + o 3 = 8 banks
        with tc.tile_pool(name="ps_s", bufs=2, space="PSUM") as ps_p, \
             tc.tile_pool(name="ps_o", bufs=1, space="PSUM") as po_p:

            def vproj(t):
                """Key-block chunk t (128 tokens) of the V projection."""
                ps = pm.tile([128, DPC], dt.float32, tag="misc", name=f"pv{t}")
                qt, ts_ = divmod(t, 4)
                for kk in range(KC):
                    nc.tensor.matmul(
                        ps,
                        lhsT=vinq[qt][:, kk * 512 + ts_ * 128
                                       : kk * 512 + (ts_ + 1) * 128],
                        rhs=w_sb["wv"][:, kk * DPC : (kk + 1) * DPC],
                        start=(kk == 0),
                        stop=(kk == KC - 1),
                    )
                vg = vaug_t[t]
                # both heads' 64 v-cols in one strided Act copy (Act may
                # read PSUM; GpSimd may not); ones memset on GpSimd (SBUF)
                dst = vg[:, 0 : 2 * VA].rearrange("p (a d) -> p a d", a=2)[
                    :, :, 0:DK
                ]
                nc.scalar.activation(
                    dst, ps.rearrange("p (a d) -> p a d", a=2), AF.Copy
                )
                ones = bass_mod.AP(
                    tensor=vg.tensor, offset=vg.offset + DK,
                    ap=[vg.ap[0], [VA, 2]],
                )
                nc.gpsimd.memset(ones, 1.0)

            def pv_mms(h, j, et, o_ps):
                for ic in range(JC):
                    nc.tensor.matmul(
                        o_ps[:, _oslc(ic) : _oslc(ic) + VA],
                        lhsT=et[:, ic * 128 : (ic + 1) * 128],
                        rhs=vaug_t[j][:, h * VA : (h + 1) * VA],
                        start=(j == 0 and ic % 7 == 0),
                        stop=(j == JC - 1 and (ic % 7 == 6 or ic == JC - 1)),
                    )

            for h in range(HPC):
                hs = h * DK
                o_ps = po_p.tile([128, 1536], dt.float32, tag="ops")
                pend = deque()
                for j in range(JC):
                    if h == 0 and j in (3, 7, 11):
                        kproj((j - 3) // 4 + 1)
                    if h == 1:
                        emit_tp(0, j, ot_h0, "misc")
                    sm = sm_p.tile([128, S], dt.bfloat16, tag="sm")
                    # GPSIMD cannot touch PSUM on real TRN2, so the whole
                    # mask-multiply rides DVE as two [128,1024] ops (bigger
                    # ops amortize the PSUM-access init cycles)
                    for half in range(2):
                        s_ps = ps_p.tile([128, 1024], dt.float32, tag="sps",
                                         name=f"s{half}_{h}_{j}")
                        for qq in range(2):
                            q = half * 2 + qq
                            nc.tensor.matmul(
                                s_ps[:, qq * 512 : (qq + 1) * 512],
                                lhsT=kT_p[j // 4][hs : hs + DK,
                                                  (j % 4) * 128 : (j % 4 + 1) * 128],
                                rhs=qT_p[q][hs : hs + DK, :],
                                start=True,
                                stop=True,
                            )
                        c = half * 1024
                        nc.vector.tensor_mul(
                            sm[:, c : c + 1024], s_ps,
                            mask_sb[:, j * S + c : j * S + c + 1024],
                        )
                    et = e_p.tile([128, S], dt.bfloat16, tag="et")
                    nc.scalar.activation(et, sm, AF.Exp, scale=1.0 / math.sqrt(DK))
                    # software pipeline (depth 2): PE emits S(j+1), S(j+2)
                    # before PV(j) so the S->TT->exp->PV chain never
                    # serializes per j, and V-quarter DMAs get extra slack.
                    pend.append((j, et))
                    if len(pend) > 2:
                        jj, ee = pend.popleft()
                        if h == 0:
                            vproj(jj)
                        pv_mms(h, jj, ee, o_ps)
                while pend:
                    jj, ee = pend.popleft()
                    if h == 0:
                        vproj(jj)
                    pv_mms(h, jj, ee, o_ps)
                # epilogue: per PSUM bank (7 ic-slices), batch-reciprocal
                # the denominator columns and batch-normalize via a 3D
                # strided AP with the recip broadcast (step-0) over DK
                ot_big = ot_p.tile([128, JC * DK], dt.bfloat16, tag="ot")

                def norm_bank(b):
                    n_ic = (7, 7, 2)[b]
                    rc = rc_p.tile([128, 8], dt.float32, tag="rc", name=f"rc{h}_{b}")
                    den = bass_mod.AP(
                        tensor=o_ps.tensor,
                        offset=o_ps.offset + b * 512 + DK,
                        ap=[o_ps.ap[0], [VA, n_ic]],
                    )
                    nc.vector.reciprocal(rc[:, :n_ic], den)
                    if h == 0 and b == 0:
                        # bank 0 normalizes on Act per-ic, inside Act's
                        # natural boundary gap (waiting for h1's first TT);
                        # sheds 0.7us from DVE's critical stream
                        for r in range(n_ic):
                            sl = bass_mod.AP(
                                tensor=o_ps.tensor,
                                offset=o_ps.offset + r * VA,
                                ap=[o_ps.ap[0], [1, DK]],
                            )
                            nc.scalar.activation(
                                ot_big[:, r * DK : (r + 1) * DK], sl,
                                AF.Copy, scale=rc[:, r : r + 1],
                            )
                        return
                    src_ap = bass_mod.AP(
                        tensor=o_ps.tensor,
                        offset=o_ps.offset + b * 512,
                        ap=[o_ps.ap[0], [VA, n_ic], [1, DK]],
                    )
                    rcb = bass_mod.AP(
                        tensor=rc.tensor,
                        offset=rc.offset,
                        ap=[rc.ap[0], [1, n_ic], [0, DK]],
                    )
                    dst = ot_big[:, b * 7 * DK : (b * 7 + n_ic) * DK].rearrange(
                        "p (a d) -> p a d", d=DK
                    )
                    nc.vector.tensor_mul(dst, src_ap, rcb)

                if h == 0:
                    for b in range(3):
                        norm_bank(b)
                def emit_tp(hh, ic, ob, psum_tag):
                    ot = ob[:, ic * DK : (ic + 1) * DK]
                    if psum_tag == "sps":
                        tp = ps_p.tile([DK, 128], dt.bfloat16, tag="sps",
                                       name=f"tp{hh}_{ic}")
                    else:
                        tp = pm.tile([DK, 128], dt.bfloat16, tag="misc",
                                     name=f"tp{hh}_{ic}")
                    nc.tensor.transpose(tp, ot, ident)
                    dst = oT_sb[ic // 4][hh * DK : hh * DK + DK,
                                         (ic % 4) * 128 : (ic % 4 + 1) * 128]
                    nc.scalar.activation(dst, tp, AF.Copy)

                if h == 0:
                    # transposes deferred into the h1 loop (misc bank is idle
                    # there); only recip+normalize touch o_ps here
                    ot_h0 = ot_big
                    continue
                for b in range(3):
                    norm_bank(b)
                ot_h1 = ot_big

        # ---- tail: transposes + output projection in dedicated deep PSUM
        # pools (the attention pools are closed, freeing 7 banks) ----
        with tc.tile_pool(name="ps_tp", bufs=4, space="PSUM") as tp_p2, \
             tc.tile_pool(name="ps_y", bufs=3, space="PSUM") as py_p:
            for ic in range(JC):
                ot = ot_h1[:, ic * DK : (ic + 1) * DK]
                tp = tp_p2.tile([DK, 128], dt.bfloat16, tag="tp",
                                name=f"tpz_{ic}")
                nc.tensor.transpose(tp, ot, ident)
                dst = oT_sb[ic // 4][DK : 2 * DK,
                                     (ic % 4) * 128 : (ic % 4 + 1) * 128]
                if ic % 2 == 0:
                    nc.scalar.activation(dst, tp, AF.Copy)
                else:
                    nc.vector.tensor_copy(dst, tp)
                if ic % 4 == 3:
                    p = ic // 4
                    for nn in range(KC):
                        y_ps = py_p.tile(
                            [128, 512], dt.float32, tag="y",
                            name=f"y{p}_{nn}"
                        )
                        nc.tensor.matmul(
                            y_ps,
                            lhsT=wo_sb[:, nn * 128 : (nn + 1) * 128],
                            rhs=oT_sb[p],
                            start=True,
                            stop=True,
                        )
                        dst = y_sb[nn][:, p * 512 : (p + 1) * 512]
                        if nn % 2 == 0:
                            nc.scalar.activation(dst, y_ps, AF.Copy)
                        else:
                            nc.vector.tensor_copy(dst, y_ps)
                        if p == 1:
                            nc.sync.dma_start(
                                out=yT_d[nn * 128 : (nn + 1) * 128, 0:1024],
                                in_=y_sb[nn][:, 0:1024],
                            )
                        elif p == TP - 1:
                            nc.sync.dma_start(
                                out=yT_d[nn * 128 : (nn + 1) * 128, 1024:2048],
                                in_=y_sb[nn][:, 1024:2048],
                            )

    nc.compile()
    return nc


def get_program():
    if "nc" not in _CACHE:
        _CACHE["nc"] = _build_program()
    return _CACHE["nc"]


def _wshuf(wT):
    """[1024 k, 128 n] -> [128 p, KC*128] with chunk kk at cols kk*128."""
    return np.ascontiguousarray(
        wT.reshape(KC, 128, DPC).transpose(1, 0, 2).reshape(128, KC * DPC)
    ).astype(BF16)


def make_in_maps(query, key, value, attention_mask, Wq, bq, Wk, bk, Wv, Wo):
    """Host-side sharding: per-core input dicts."""
    qT = np.ascontiguousarray(np.asarray(query, np.float32)[0].T).astype(BF16)
    kT = np.ascontiguousarray(np.asarray(key, np.float32)[0].T).astype(BF16)
    vT = np.ascontiguousarray(np.asarray(value, np.float32)[0].T).astype(BF16)
    maskT = np.ascontiguousarray(
        np.asarray(attention_mask, np.float32)[0, 0].T
    ).astype(FP8)

    in_maps = []
    for c in range(NCORES):
        ns = slice(c * DPC, (c + 1) * DPC)
        in_maps.append(
            {
                "qT": qT,
                "kT": kT,
                "vT": vT,
                "maskT": maskT,
                "wq": _wshuf(np.asarray(Wq, np.float32)[ns].T),
                "wk": _wshuf(np.asarray(Wk, np.float32)[ns].T),
                "wv": _wshuf(np.asarray(Wv, np.float32)[ns].T),
                "wo": np.ascontiguousarray(np.asarray(Wo, np.float32)[:, ns].T).astype(BF16),
                "bq": np.asarray(bq, np.float32)[ns, None],
                "bk": np.asarray(bk, np.float32)[ns, None],
                "ident": np.eye(128, dtype=BF16),
            }
        )
    return in_maps


def combine_outputs(results, Wv_bias, Wo, bo):
    """Sum per-core partial yT's (bf16 -> fp32), add host-folded biases."""
    acc = np.zeros((H, S), np.float32)
    for r in results:
        acc += r["yT"].astype(np.float32)
    bias = np.asarray(bo, np.float32) + np.asarray(Wv_bias, np.float32) @ np.asarray(
        Wo, np.float32
    ).T
    return (acc.T + bias[None, :]).astype(np.float32)[None]


def kernel(
    query,
    key,
    value,
    attention_mask,
    Wq,
    bq,
    Wk,
    bk,
    Wv,
    bv,
    Wo,
    bo,
    head,
    hidden_size,
):
    from concourse.bass_utils import run_bass_kernel_spmd

    nc = get_program()
    in_maps = make_in_maps(
        query, key, value, attention_mask, Wq, bq, Wk, bk, Wv, Wo
    )
    res = run_bass_kernel_spmd(nc, in_maps, list(range(NCORES)))
    return combine_outputs(res.results, bv, Wo, bo)


# revision 69
# speedup vs baseline: 1.0155x; 1.0155x over previous
"""Multi-head attention (B=1, S=2048, H=1024, NH=16) on 8 trn2 NeuronCores.

Sharding: head-parallel. Core c owns heads {2c, 2c+1} (= 128 of the 1024
hidden dims). Each core computes its Q/K/V projection slices, the full
attention for its 2 heads, and a full-width partial of the output
projection (contraction over its 128 context dims). Host sums the 8
partials and adds the (host-folded) biases.

Schedule notes (from TimelineSim iteration; ~134us modeled vs 157us v1):
  - Real TRN2 GpSimd cannot access PSUM, and nearly all elementwise work
    here reads PSUM, so DVE carries the mask multiplies (2x [128,1024]
    TensorTensor per (h,j), ~2.4us/iter = the cadence) and Act carries
    exp plus most PSUM->SBUF evictions in its gaps. GpSimd only memsets.
  - Writes from different engines into one tile serialize (tile-level
    WAW), so every multi-writer tile is split: qT/kT per 512-panel,
    vaug per key-chunk, sm/et single-writer.
  - Q/K biases fold into the eviction (Act Identity+bias / DVE
    tensor_scalar_add); no bias matmuls.
  - K projection in 4 token-panels: p0 prefetched inside the Q-proj DMA
    gaps (PSUM misc bank spans both phases), p1-3 interleaved at j=3,7,11.
  - PE pstate warmup: junk matmuls on a memset tile keep the dispatch
    stream dense through the DMA-gated projection so the model's p-state
    stays at 2.4GHz.
  - DMAs are emitted in deadline order (mask in j-pair chunks, V in
    quarter-chunks) because the 16.3MB input stream (~47us) paces h0.
  - h0's transposes are deferred into the h1 loop; h1's transposes and
    the output projection run after the attention pools close, in
    dedicated PSUM pools (tp x4 banks, y x3) so the MM->evict chain
    pipelines; output DMA per (nn, seq-half).

Precision: all matmuls bf16 with fp32 PSUM accumulation. The 0/1 mask is
stored fp8-e4m3 (exact, halves its bandwidth). Softmax runs without
max-subtraction: the exponent is (q.k/8)*M ~ N(0, 0.33^2), so exp never
overflows.
"""

import math
from collections import deque

import numpy as np
import ml_dtypes

BF16 = ml_dtypes.bfloat16
FP8 = ml_dtypes.float8_e4m3
S, H, NH, DK = 2048, 1024, 16, 64
NCORES = 8
HPC = NH // NCORES          # heads per core = 2
DPC = HPC * DK              # head dims per core = 128
KC = H // 128               # contraction chunks = 8
TP = S // 512               # 512-wide token panels = 4
JC = S // 128               # 128-wide key chunks = 16
VA = DK + 1                 # v columns + ones column = 65

_CACHE = {}


def _oslc(ic):
    """o_ps column offset for ic-th 65-wide slice: 7 slices per 512-fp32
    PSUM bank so no matmul crosses a bank boundary."""
    b, r = divmod(ic, 7)
    return b * 512 + r * VA


def _build_program():
    """Build + compile the (identical) per-core Bass program."""
    from contextlib import ExitStack

    import concourse.bacc as bacc
    import concourse.tile as tile
    from concourse import mybir
    import concourse.bass as bass_mod

    dt = mybir.dt
    AF = mybir.ActivationFunctionType
    f8 = dt.float8e4

    nc = bacc.Bacc("TRN2", target_bir_lowering=False, debug=False)

    qT_d = nc.dram_tensor("qT", [H, S], dt.bfloat16, kind="ExternalInput").ap()
    kT_d = nc.dram_tensor("kT", [H, S], dt.bfloat16, kind="ExternalInput").ap()
    vT_d = nc.dram_tensor("vT", [H, S], dt.bfloat16, kind="ExternalInput").ap()
    maskT_d = nc.dram_tensor("maskT", [S, S], f8, kind="ExternalInput").ap()
    wq_d = nc.dram_tensor("wq", [128, KC * DPC], dt.bfloat16, kind="ExternalInput").ap()
    wk_d = nc.dram_tensor("wk", [128, KC * DPC], dt.bfloat16, kind="ExternalInput").ap()
    wv_d = nc.dram_tensor("wv", [128, KC * DPC], dt.bfloat16, kind="ExternalInput").ap()
    wo_d = nc.dram_tensor("wo", [DPC, H], dt.bfloat16, kind="ExternalInput").ap()
    bq_d = nc.dram_tensor("bq", [DPC, 1], dt.float32, kind="ExternalInput").ap()
    bk_d = nc.dram_tensor("bk", [DPC, 1], dt.float32, kind="ExternalInput").ap()
    id_d = nc.dram_tensor("ident", [128, 128], dt.bfloat16, kind="ExternalInput").ap()
    yT_d = nc.dram_tensor("yT", [H, S], dt.bfloat16, kind="ExternalOutput").ap()

    with tile.TileContext(nc) as tc, ExitStack() as ctx:
        cp = ctx.enter_context(tc.tile_pool(name="const", bufs=1))
        sm_p = ctx.enter_context(tc.tile_pool(name="sm", bufs=3))
        e_p = ctx.enter_context(tc.tile_pool(name="ex", bufs=4))
        ot_p = ctx.enter_context(tc.tile_pool(name="otok", bufs=2))
        rc_p = ctx.enter_context(tc.tile_pool(name="recip", bufs=3))
        w_sb = {}
        for name, d in (("wq", wq_d), ("wk", wk_d), ("wv", wv_d)):
            w_sb[name] = cp.tile([128, KC * DPC], dt.bfloat16, tag=name, name=name)
        wo_sb = cp.tile([128, H], dt.bfloat16, tag="wo")
        bq_sb = cp.tile([DPC, 1], dt.float32, tag="bq")
        bk_sb = cp.tile([DPC, 1], dt.float32, tag="bk")
        qT_p = [cp.tile([128, 512], dt.bfloat16, tag=f"qT{p}", name=f"qT{p}")
                for p in range(TP)]
        kT_p = [cp.tile([128, 512], dt.bfloat16, tag=f"kT{p}", name=f"kT{p}")
                for p in range(TP)]
        vaug_t = [cp.tile([128, HPC * VA], dt.bfloat16, tag=f"vg{t}",
                          name=f"vg{t}") for t in range(JC)]
        ident = cp.tile([128, 128], dt.bfloat16, tag="ident")
        oT_sb = [cp.tile([128, 512], dt.bfloat16, tag=f"oTp{p}", name=f"oTp{p}")
                 for p in range(TP)]
        mask_sb = cp.tile([128, JC * S], f8, tag="mask")
        kin = [cp.tile([128, KC * 512], dt.bfloat16, tag=f"kin{p}", name=f"kin{p}")
               for p in range(TP)]
        vinq = [cp.tile([128, KC * 512], dt.bfloat16, tag=f"vin{qt}",
                        name=f"vin{qt}") for qt in range(4)]
        y_sb = [cp.tile([128, S], dt.bfloat16, tag=f"ysb{nn}", name=f"ysb{nn}")
                for nn in range(KC)]

        def dma_mask2(j2):
            nc.sync.dma_start(
                out=mask_sb[:, j2 * 2 * S : (j2 + 1) * 2 * S].rearrange(
                    "p (a i) -> p a i", a=2
                ),
                in_=maskT_d[j2 * 256 : (j2 + 1) * 256, :].rearrange(
                    "(a p) i -> p a i", p=128
                ),
            )

        def dma_kin(p):
            nc.sync.dma_start(
                out=kin[p].rearrange("p (c i) -> p c i", c=KC),
                in_=kT_d[:, p * 512 : (p + 1) * 512].rearrange(
                    "(c p) i -> p c i", p=128
                ),
            )

        def dma_vq(qt):
            nc.sync.dma_start(
                out=vinq[qt].rearrange("p (c i) -> p c i", c=KC),
                in_=vT_d[:, qt * 512 : (qt + 1) * 512].rearrange(
                    "(c p) i -> p c i", p=128
                ),
            )

        # ---- Q projection (+ all input DMAs, emitted in deadline order) ----
        pm = ctx.enter_context(tc.tile_pool(name="ps_misc", bufs=1, space="PSUM"))

        def kproj_mms(ps, p, kks):
            for kk in kks:
                nc.tensor.matmul(
                    ps,
                    lhsT=w_sb["wk"][:, kk * DPC : (kk + 1) * DPC],
                    rhs=kin[p][:, kk * 512 : (kk + 1) * 512],
                    start=(kk == 0),
                    stop=(kk == KC - 1),
                )

        def kproj(p):
            ps = pm.tile([128, 512], dt.float32, tag="misc", name=f"pk{p}")
            kproj_mms(ps, p, range(KC))
            nc.scalar.activation(kT_p[p], ps, AF.Identity, bias=bk_sb)

        with tc.tile_pool(name="ps_q", bufs=1, space="PSUM") as pq, \
             tc.tile_pool(name="xin", bufs=3) as xin_p:
            nc.sync.dma_start(out=w_sb["wq"], in_=wq_d)
            nc.sync.dma_start(out=w_sb["wk"], in_=wk_d)
            xq = []

            def dma_xq(hf):
                for cc in range(2):
                    xt = xin_p.tile([128, S], dt.bfloat16, tag="xq",
                                    name=f"xq{2 * hf + cc}")
                    nc.sync.dma_start(
                        out=xt,
                        in_=qT_d[hf * 256 + cc * 128
                                 : hf * 256 + (cc + 1) * 128, :],
                    )
                    xq.append(xt)

            dma_xq(0)
            dma_kin(0)
            dma_xq(1)
            dma_mask2(0)
            nc.sync.dma_start(out=bq_sb, in_=bq_d)
            nc.sync.dma_start(out=bk_sb, in_=bk_d)
            dma_xq(2)
            dma_xq(3)
            nc.sync.dma_start(out=w_sb["wv"], in_=wv_d)
            dma_vq(0)
            dma_mask2(1)
            dma_kin(1)
            dma_vq(1)
            dma_mask2(2)
            nc.sync.dma_start(out=ident, in_=id_d)
            dma_mask2(3)
            dma_kin(2)
            dma_mask2(4)
            dma_vq(2)
            dma_mask2(5)
            dma_kin(3)
            dma_mask2(6)
            dma_vq(3)
            dma_mask2(7)
            nc.sync.dma_start(out=wo_sb, in_=wo_d)

            psl = [pq.tile([128, 512], dt.float32, tag=f"pq{p}", name=f"pq{p}")
                   for p in range(TP)]
            warm_in = cp.tile([128, 256], dt.bfloat16, tag="warm")
            nc.vector.memset(warm_in, 0.0)
            junk = pq.tile([128, 256], dt.float32, tag="junk")

            def warm(n):
                for _ in range(n):
                    nc.tensor.matmul(
                        junk, lhsT=warm_in[:, 0:128], rhs=warm_in,
                        start=True, stop=True,
                    )

            warm(40)
            for kk in range(KC):
                for p in range(TP):
                    nc.tensor.matmul(
                        psl[p],
                        lhsT=w_sb["wq"][:, kk * DPC : (kk + 1) * DPC],
                        rhs=xq[kk][:, p * 512 : (p + 1) * 512],
                        start=(kk == 0),
                        stop=(kk == KC - 1),
                    )
                if kk % 2 == 1 and kk < 7:
                    warm(6)
                if kk == 3:
                    kproj(0)
            for p in range(TP):
                if p % 2 == 0:
                    nc.scalar.activation(
                        qT_p[p], psl[p], AF.Identity, bias=bq_sb,
                    )
                else:
                    nc.vector.tensor_scalar_add(qT_p[p], psl[p], bq_sb)

        # ---- attention (K panels + V projection interleaved) ----
        # PSUM: misc (kproj/vproj/tp) 1 bank + s 2x2 + o 3 = 8 banks
        with tc.tile_pool(name="ps_s", bufs=2, space="PSUM") as ps_p, \
             tc.tile_pool(name="ps_o", bufs=1, space="PSUM") as po_p:

            def vproj(t):
                """Key-block chunk t (128 tokens) of the V projection."""
                ps = pm.tile([128, DPC], dt.float32, tag="misc", name=f"pv{t}")
                qt, ts_ = divmod(t, 4)
                for kk in range(KC):
                    nc.tensor.matmul(
                        ps,
                        lhsT=vinq[qt][:, kk * 512 + ts_ * 128
                                       : kk * 512 + (ts_ + 1) * 128],
                        rhs=w_sb["wv"][:, kk * DPC : (kk + 1) * DPC],
                        start=(kk == 0),
                        stop=(kk == KC - 1),
                    )
                vg = vaug_t[t]
                # both heads' 64 v-cols in one strided Act copy (Act may
                # read PSUM; GpSimd may not); ones memset on GpSimd (SBUF)
                dst = vg[:, 0 : 2 * VA].rearrange("p (a d) -> p a d", a=2)[
                    :, :, 0:DK
                ]
                nc.scalar.activation(
                    dst, ps.rearrange("p (a d) -> p a d", a=2), AF.Copy
                )
                ones = bass_mod.AP(
                    tensor=vg.tensor, offset=vg.offset + DK,
                    ap=[vg.ap[0], [VA, 2]],
                )
                nc.gpsimd.memset(ones, 1.0)

            def pv_mms(h, j, et, o_ps):
                for ic in range(JC):
                    nc.tensor.matmul(
                        o_ps[:, _oslc(ic) : _oslc(ic) + VA],
                        lhsT=et[:, ic * 128 : (ic + 1) * 128],
                        rhs=vaug_t[j][:, h * VA : (h + 1) * VA],
                        start=(j == 0 and ic % 7 == 0),
                        stop=(j == JC - 1 and (ic % 7 == 6 or ic == JC - 1)),
                    )

            for h in range(HPC):
                hs = h * DK
                o_ps = po_p.tile([128, 1536], dt.float32, tag="ops")
                pend = deque()
                for j in range(JC):
                    if h == 0 and j in (3, 7, 11):
                        kproj((j - 3) // 4 + 1)
                    if h == 1:
                        emit_tp(0, j, ot_h0, "misc")
                    sm = sm_p.tile([128, S], dt.bfloat16, tag="sm")
                    # GPSIMD cannot touch PSUM on real TRN2, so the whole
                    # mask-multiply rides DVE as two [128,1024] ops (bigger
                    # ops amortize the PSUM-access init cycles)
                    for half in range(2):
                        s_ps = ps_p.tile([128, 1024], dt.float32, tag="sps",
                                         name=f"s{half}_{h}_{j}")
                        for qq in range(2):
                            q = half * 2 + qq
                            nc.tensor.matmul(
                                s_ps[:, qq * 512 : (qq + 1) * 512],
                                lhsT=kT_p[j // 4][hs : hs + DK,
                                                  (j % 4) * 128 : (j % 4 + 1) * 128],
                                rhs=qT_p[q][hs : hs + DK, :],
                                start=True,
                                stop=True,
                            )
                        c = half * 1024
                        nc.vector.tensor_mul(
                            sm[:, c : c + 1024], s_ps,
                            mask_sb[:, j * S + c : j * S + c + 1024],
                        )
                    et = e_p.tile([128, S], dt.bfloat16, tag="et")
                    nc.scalar.activation(et, sm, AF.Exp, scale=1.0 / math.sqrt(DK))
                    # software pipeline (depth 2): PE emits S(j+1), S(j+2)
                    # before PV(j) so the S->TT->exp->PV chain never
                    # serializes per j, and V-quarter DMAs get extra slack.
                    pend.append((j, et))
                    if len(pend) > 2:
                        jj, ee = pend.popleft()
                        if h == 0:
                            vproj(jj)
                        pv_mms(h, jj, ee, o_ps)
                while pend:
                    jj, ee = pend.popleft()
                    if h == 0:
                        vproj(jj)
                    pv_mms(h, jj, ee, o_ps)
                # epilogue: per PSUM bank (7 ic-slices), batch-reciprocal
                # the denominator columns and batch-normalize via a 3D
                # strided AP with the recip broadcast (step-0) over DK
                ot_big = ot_p.tile([128, JC * DK], dt.bfloat16, tag="ot")

                def norm_bank(b):
                    n_ic = (7, 7, 2)[b]
                    rc = rc_p.tile([128, 8], dt.float32, tag="rc", name=f"rc{h}_{b}")
                    den = bass_mod.AP(
                        tensor=o_ps.tensor,
                        offset=o_ps.offset + b * 512 + DK,
                        ap=[o_ps.ap[0], [VA, n_ic]],
                    )
                    nc.vector.reciprocal(rc[:, :n_ic], den)
                    if h == 0 and b == 2:
                        for r in range(n_ic):
                            sl = bass_mod.AP(
                                tensor=o_ps.tensor,
                                offset=o_ps.offset + b * 512 + r * VA,
                                ap=[o_ps.ap[0], [1, DK]],
                            )
                            nc.scalar.activation(
                                ot_big[:, (b * 7 + r) * DK
                                       : (b * 7 + r + 1) * DK], sl,
                                AF.Copy, scale=rc[:, r : r + 1],
                            )
                        return
                    src_ap = bass_mod.AP(
                        tensor=o_ps.tensor,
                        offset=o_ps.offset + b * 512,
                        ap=[o_ps.ap[0], [VA, n_ic], [1, DK]],
                    )
                    rcb = bass_mod.AP(
                        tensor=rc.tensor,
                        offset=rc.offset,
                        ap=[rc.ap[0], [1, n_ic], [0, DK]],
                    )
                    dst = ot_big[:, b * 7 * DK : (b * 7 + n_ic) * DK].rearrange(
                        "p (a d) -> p a d", d=DK
                    )
                    nc.vector.tensor_mul(dst, src_ap, rcb)

                if h == 0:
                    for b in range(3):
                        norm_bank(b)
                def emit_tp(hh, ic, ob, psum_tag):
                    ot = ob[:, ic * DK : (ic + 1) * DK]
                    if psum_tag == "sps":
                        tp = ps_p.tile([DK, 128], dt.bfloat16, tag="sps",
                                       name=f"tp{hh}_{ic}")
                    else:
                        tp = pm.tile([DK, 128], dt.bfloat16, tag="misc",
                                     name=f"tp{hh}_{ic}")
                    nc.tensor.transpose(tp, ot, ident)
                    dst = oT_sb[ic // 4][hh * DK : hh * DK + DK,
                                         (ic % 4) * 128 : (ic % 4 + 1) * 128]
                    nc.scalar.activation(dst, tp, AF.Copy)

                if h == 0:
                    # transposes deferred into the h1 loop (misc bank is idle
                    # there); only recip+normalize touch o_ps here
                    ot_h0 = ot_big
                    continue
                for b in range(3):
                    norm_bank(b)
                ot_h1 = ot_big

        # ---- tail: transposes + output projection in dedicated deep PSUM
        # pools (the attention pools are closed, freeing 7 banks) ----
        with tc.tile_pool(name="ps_tp", bufs=4, space="PSUM") as tp_p2, \
             tc.tile_pool(name="ps_y", bufs=3, space="PSUM") as py_p:
            for ic in range(JC):
                ot = ot_h1[:, ic * DK : (ic + 1) * DK]
                tp = tp_p2.tile([DK, 128], dt.bfloat16, tag="tp",
                                name=f"tpz_{ic}")
                nc.tensor.transpose(tp, ot, ident)
                dst = oT_sb[ic // 4][DK : 2 * DK,
                                     (ic % 4) * 128 : (ic % 4 + 1) * 128]
                if ic % 2 == 0:
                    nc.scalar.activation(dst, tp, AF.Copy)
                else:
                    nc.vector.tensor_copy(dst, tp)
                if ic % 4 == 3:
                    p = ic // 4
                    for nn in range(KC):
                        y_ps = py_p.tile(
                            [128, 512], dt.float32, tag="y",
                            name=f"y{p}_{nn}"
                        )
                        nc.tensor.matmul(
                            y_ps,
                            lhsT=wo_sb[:, nn * 128 : (nn + 1) * 128],
                            rhs=oT_sb[p],
                            start=True,
                            stop=True,
                        )
                        dst = y_sb[nn][:, p * 512 : (p + 1) * 512]
                        if nn % 2 == 0:
                            nc.scalar.activation(dst, y_ps, AF.Copy)
                        else:
                            nc.vector.tensor_copy(dst, y_ps)
                        if p == 1:
                            nc.sync.dma_start(
                                out=yT_d[nn * 128 : (nn + 1) * 128, 0:1024],
                                in_=y_sb[nn][:, 0:1024],
                            )
                        elif p == TP - 1:
                            nc.sync.dma_start(
                                out=yT_d[nn * 128 : (nn + 1) * 128, 1024:2048],
                                in_=y_sb[nn][:, 1024:2048],
                            )

    nc.compile()
    return nc


def get_program():
    if "nc" not in _CACHE:
        _CACHE["nc"] = _build_program()
    return _CACHE["nc"]


def _wshuf(wT):
    """[1024 k, 128 n] -> [128 p, KC*128] with chunk kk at cols kk*128."""
    return np.ascontiguousarray(
        wT.reshape(KC, 128, DPC).transpose(1, 0, 2).reshape(128, KC * DPC)
    ).astype(BF16)


def make_in_maps(query, key, value, attention_mask, Wq, bq, Wk, bk, Wv, Wo):
    """Host-side sharding: per-core input dicts."""
    qT = np.ascontiguousarray(np.asarray(query, np.float32)[0].T).astype(BF16)
    kT = np.ascontiguousarray(np.asarray(key, np.float32)[0].T).astype(BF16)
    vT = np.ascontiguousarray(np.asarray(value, np.float32)[0].T).astype(BF16)
    maskT = np.ascontiguousarray(
        np.asarray(attention_mask, np.float32)[0, 0].T
    ).astype(FP8)

    in_maps = []
    for c in range(NCORES):
        ns = slice(c * DPC, (c + 1) * DPC)
        in_maps.append(
            {
                "qT": qT,
                "kT": kT,
                "vT": vT,
                "maskT": maskT,
                "wq": _wshuf(np.asarray(Wq, np.float32)[ns].T),
                "wk": _wshuf(np.asarray(Wk, np.float32)[ns].T),
                "wv": _wshuf(np.asarray(Wv, np.float32)[ns].T),
                "wo": np.ascontiguousarray(np.asarray(Wo, np.float32)[:, ns].T).astype(BF16),
                "bq": np.asarray(bq, np.float32)[ns, None],
                "bk": np.asarray(bk, np.float32)[ns, None],
                "ident": np.eye(128, dtype=BF16),
            }
        )
    return in_maps


def combine_outputs(results, Wv_bias, Wo, bo):
    """Sum per-core partial yT's (bf16 -> fp32), add host-folded biases."""
    acc = np.zeros((H, S), np.float32)
    for r in results:
        acc += r["yT"].astype(np.float32)
    bias = np.asarray(bo, np.float32) + np.asarray(Wv_bias, np.float32) @ np.asarray(
        Wo, np.float32
    ).T
    return (acc.T + bias[None, :]).astype(np.float32)[None]


def kernel(
    query,
    key,
    value,
    attention_mask,
    Wq,
    bq,
    Wk,
    bk,
    Wv,
    bv,
    Wo,
    bo,
    head,
    hidden_size,
):
    from concourse.bass_utils import run_bass_kernel_spmd

    nc = get_program()
    in_maps = make_in_maps(
        query, key, value, attention_mask, Wq, bq, Wk, bk, Wv, Wo
    )
    res = run_bass_kernel_spmd(nc, in_maps, list(range(NCORES)))
    return combine_outputs(res.results, bv, Wo, bo)
